# revision 1
# baseline (speedup 1.0000x reference)
"""Trainium2 Bass kernel for nn_Map_79748952752358 (dense_cnn).

Pipeline per sample batch: LSTM encoder (40 steps) -> e2d projection ->
big linear (lin1: 256 -> 262144) -> per-sample dynamic 1x1 conv over
feature [1024, 32x32] -> BN(eval) -> channel-max -> clip.

Sharding: 8-way over the R=256 conv output channels (32 per core). Every
core runs the full LSTM + e2d (replicated), computes its 32-row slice of
the dynamic filters (lin1 tensor-sharded over rows), convolves against
the full feature tensor, and emits a per-core partial channel-max
[16, 1024].  The host combines partials with np.maximum, applies the
BN-shift floor T0 = max_r(t_r) and the final clip.

Key math folds (exact, done on host):
  - BN scale s=gamma/sqrt(var+eps) > 0 folded into lin1 weights/bias
    (LeakyRelu and Relu are positively homogeneous).
  - relu(x)+t maxed over r == max(max_r(x+t), max_r(t)); the +t is
    injected into the conv PSUM via a rank-1 matmul, max_r(t) on host.
"""

import os
import numpy as np
import ml_dtypes

import concourse.bass as bass
import concourse.mybir as mybir
from concourse import tile
from concourse.tile import ScopedClock
from concourse.alu_op_type import AluOpType
from concourse.bass_utils import run_bass_kernel_spmd

BF16 = ml_dtypes.bfloat16

B, S, V, E, HID = 16, 40, 1004, 256, 256
C, R, HW2 = 1024, 256, 1024
BN_EPS = 1e-5
N_CORES = 8
RS = R // N_CORES  # 32 r-rows per core
P = 128

AFT = mybir.ActivationFunctionType
AX = mybir.AxisListType


# ---------------------------------------------------------------------------
# Tile tail-drain patch: this walrus build accepts fewer sem waits per
# TPB_CTRL instruction than Tile's exit drain accumulates; split them into
# single-wait SP nops.
_drain_patched = False


def _patch_tile_drain():
    global _drain_patched
    if _drain_patched:
        return
    _drain_patched = True

    def _patched(self, tick_clock, wait_clock):
        nc = self.nc
        probe = nc.sync.nop(nofuse=True, hint="drain_wait_split")
        wait_clock.add_sem_waits(
            probe.ins, ScopedClock({None: tick_clock.global_clock})
        )
        si = probe.ins.sync_info
        waits = list(si.on_wait or []) if si is not None else []
        if len(waits) > 1:
            si.on_wait = waits[:1]
            for w in waits[1:]:
                n = nc.sync.nop(nofuse=True, hint="drain_wait_split")
                nsi = n.ins.sync_info
                if nsi is None:
                    import bass_rust

                    n.ins.sync_info = bass_rust.SyncInfo(on_wait=[w], on_update=[])
                else:
                    nsi.on_wait = [w]
        nc.sync.drain()
        nc.all_engine_barrier()
        assert self.sems is not None
        popped = nc._tile_sem_poison_stack.pop()
        assert popped is self._sem_poison
        nc.clear_and_free_semaphores(list(self.sems.allocated().values()))
        nc.all_engine_barrier()

    tile.TileContext._drain_and_barrier = _patched


_ws_counter = [0]


def _split_excess_waits(nc, limit=1):
    """Walrus on this image rejects instructions with more than ~2 sem waits.
    Move excess waits onto same-engine EventSemaphore carriers inserted just
    before the offending instruction (same per-engine stream order, identical
    blocking semantics)."""
    import bass_rust

    for fn in nc.m.functions:
        for bb in fn.blocks:
            out = []
            for inst in bb.instructions:
                si = inst.sync_info
                waits = list(si.on_wait or []) if si is not None else []
                if len(waits) > limit:
                    for w in waits[:-limit]:
                        _ws_counter[0] += 1
                        carrier = mybir.InstEventSemaphore(
                            name=f"I-waitsplit-{_ws_counter[0]}",
                            opcode="EventSemaphore",
                            engine=inst.engine,
                            sync_info=bass_rust.SyncInfo(
                                on_wait=[w], on_update=[]),
                        )
                        out.append(carrier)
                    si.on_wait = waits[-limit:]
                out.append(inst)
            bb.instructions = out


# ---------------------------------------------------------------------------
def _build_program(slots):
    """Build the SPMD Bass program. `slots[b]` = length[b]-1, the LSTM step
    whose hidden state is each sample's final state (compile-time constants).
    """
    _patch_tile_drain()
    nc = bass.Bass("TRN2", target_bir_lowering=False, debug=False,
                   num_devices=N_CORES)
    dt = mybir.dt
    f32, bf16 = dt.float32, dt.bfloat16

    def din(name, shape, d=bf16):
        return nc.dram_tensor(name, shape, d, kind="ExternalInput").ap()

    feat_d = din("feat", [B, P, 8, HW2])          # (b, c_in, c-tile, hw) bf16
    # one packed bf16 constant block: embT | wihT | whhT | e2dT | eye |
    # b1 | delta | tpat | ones_row  (the 32-partition blocks are zero-padded)
    PK = [2 * S * B, 16 * P, 16 * P, 4 * P, P, 8 * P, RS * B, P, HW2 // 2]
    pack_d = din("cpack", [P, sum(PK)])
    biasf_d = din("biasf", [P, 10], f32)          # biasg(8) | e2db(2)
    w1_d = din("w1T", [P, 8 * RS * 2 * P])        # tiles (ct, r, kh)

    out_d = nc.dram_tensor("part_out", [P, B * 8], f32, kind="ExternalOutput").ap()

    with tile.TileContext(nc) as tc:
        with (
            tc.tile_pool(name="const", bufs=1) as cpool,
            tc.tile_pool(name="xg", bufs=1) as xgpool,
            tc.tile_pool(name="hist", bufs=1) as hpool,
            tc.tile_pool(name="gs", bufs=2) as gspool,
            tc.tile_pool(name="cell", bufs=1) as cellpool,
            tc.tile_pool(name="tmp", bufs=4) as tmppool,
            tc.tile_pool(name="w1c", bufs=2) as w1pool,
            tc.tile_pool(name="f1", bufs=1) as f1pool,
            tc.tile_pool(name="feat", bufs=7) as fpool,
            tc.tile_pool(name="vout", bufs=1) as vpool,
        ):
            pack = cpool.tile([P, sum(PK)], bf16, tag="cpack")
            nc.sync.dma_start(out=pack[:], in_=pack_d)
            biasf = cpool.tile([P, 10], f32, tag="biasf")
            nc.sync.dma_start(out=biasf[:], in_=biasf_d)

            off = np.cumsum([0] + PK)
            embT = pack[:, off[0]:off[1]]
            wih = pack[:, off[1]:off[2]]
            whh = pack[:, off[2]:off[3]]
            e2dT = pack[:, off[3]:off[4]]
            eye = pack[:, off[4]:off[5]]
            b1 = pack[0:RS, off[5]:off[6]]
            delta = pack[0:RS, off[6]:off[7]]
            tpat = pack[0:RS, off[7]:off[8]]
            ones_row = pack[0:RS, off[8]:off[9]]
            biasg = biasf[:, 0:8]
            e2db = biasf[:, 8:10]

            # ---- Stage A: xg = w_ih @ x_t for all steps (+ gate bias) ----
            # out tiles: xg_s[p, m*640 + t*16 + b]  (bf16)
            xg_s = xgpool.tile([P, 8 * S * B], bf16)
            NCH = 320  # psum N-chunk: 20 steps x 16
            with tc.tile_pool(name="xpsum", bufs=2, space="PSUM") as xpsum:
                for m in range(8):
                    for n in range(2):
                        ps = xpsum.tile([P, NCH], f32, tag="xg")
                        for ke in range(2):
                            nc.tensor.matmul(
                                ps[:],
                                lhsT=wih[:, (ke * 8 + m) * P:(ke * 8 + m + 1) * P],
                                rhs=embT[:, ke * S * B + n * NCH: ke * S * B + (n + 1) * NCH],
                                start=(ke == 0), stop=(ke == 1),
                            )
                        nc.scalar.activation(
                            out=xg_s[:, m * S * B + n * NCH: m * S * B + (n + 1) * NCH],
                            in_=ps[:], func=AFT.Identity, bias=biasg[:, m:m + 1],
                        )

            # ---- Stage B: LSTM recurrence (layout: gate-dim on partitions) --
            S_eff = max(slots) + 1
            hist = hpool.tile([P, S * 2 * B], bf16)   # (t, kh, b)
            c_s = cellpool.tile([P, 2 * B], f32)      # (kh, b)
            xg_r = xg_s[:].rearrange("p (m t b) -> p m t b", m=8, t=S)
            lstm_psum = tc.tile_pool(name="gpsum", bufs=2, space="PSUM")
            gpsum = lstm_psum.__enter__()
            for t in range(S_eff):
                gp = gpsum.tile([P, P], f32, tag="gates")
                nc.tensor.matmul(gp[:], lhsT=eye[:], rhs=xg_r[:, :, t, :],
                                 start=True, stop=(t == 0))
                if t > 0:
                    for m in range(8):
                        for kh in range(2):
                            nc.tensor.matmul(
                                gp[:, m * B:(m + 1) * B],
                                lhsT=whh[:, (kh * 8 + m) * P:(kh * 8 + m + 1) * P],
                                rhs=hist[:, (t - 1) * 2 * B + kh * B:
                                         (t - 1) * 2 * B + (kh + 1) * B],
                                start=False, stop=(m == 7 and kh == 1),
                                skip_group_check=True,
                            )
                gs = gspool.tile([P, P], f32, tag="gs")
                # cols (m,b): i=0:32, f=32:64, g=64:96, o=96:128
                # one sigmoid for all gates; tanh(g)=2*sig(2g)-1 (g-rows
                # pre-scaled by 2 on host; fp32 to avoid rounding blowup)
                nc.scalar.activation(out=gs[:], in_=gp[:], func=AFT.Sigmoid)
                tg = tmppool.tile([P, 2 * B], f32, tag="tg")
                nc.vector.tensor_scalar(tg[:], gs[:, 64:96], 2.0, -1.0,
                                        AluOpType.mult, AluOpType.add)
                t1 = tmppool.tile([P, 2 * B], f32, tag="t1")
                nc.vector.tensor_tensor(t1[:], gs[:, 0:32], tg[:],
                                        AluOpType.mult)
                if t == 0:
                    nc.vector.tensor_copy(c_s[:], t1[:])
                else:
                    t2 = tmppool.tile([P, 2 * B], f32, tag="t2")
                    nc.vector.tensor_tensor(t2[:], gs[:, 32:64], c_s[:],
                                            AluOpType.mult)
                    nc.vector.tensor_tensor(c_s[:], t1[:], t2[:], AluOpType.add)
                th = tmppool.tile([P, 2 * B], bf16, tag="th")
                nc.scalar.activation(out=th[:], in_=c_s[:], func=AFT.Tanh)
                nc.vector.tensor_tensor(
                    hist[:, t * 2 * B:(t + 1) * 2 * B],
                    gs[:, 96:128], th[:], AluOpType.mult)

            # ---- capture final h per sample (slots known at build time) ----
            h_fin = cellpool.tile([P, 2 * B], bf16, tag="hfin")  # (kh, b)
            hf_r = h_fin[:].rearrange("p (k b) -> p b k", k=2)
            for b in range(B):
                src = hist[:, slots[b] * 2 * B:(slots[b] + 1) * 2 * B]
                nc.vector.tensor_copy(
                    hf_r[:, b], src.rearrange("p (k b) -> p b k", k=2)[:, b])

            # ---- e2d projection: instrT = tanh(e2d_w @ h + b) -------------
            instrT = cellpool.tile([P, 2 * B], bf16, tag="instrT")  # (kh, b)
            for m in range(2):
                pe2 = gpsum.tile([P, B], f32, tag="e2d")
                for kh in range(2):
                    nc.tensor.matmul(
                        pe2[:],
                        lhsT=e2dT[:, (kh * 2 + m) * P:(kh * 2 + m + 1) * P],
                        rhs=h_fin[:, kh * B:(kh + 1) * B],
                        start=(kh == 0), stop=(kh == 1),
                    )
                nc.scalar.activation(out=instrT[:, m * B:(m + 1) * B],
                                     in_=pe2[:], func=AFT.Tanh,
                                     bias=e2db[:, m:m + 1])
            lstm_psum.__exit__(None, None, None)

            # ---- lin1 (r-slice): f1T[c, (ct,b,r)] = Lrelu(W_ct_r @ instr + b1)
            f1_sb = f1pool.tile([P, 8 * B * RS], bf16)
            CW = RS * 2 * P  # w1 chunk cols per ct
            lin1_psum = tc.tile_pool(name="lpsum", bufs=4, space="PSUM")
            lpsum = lin1_psum.__enter__()
            for ct in range(8):
                wch = w1pool.tile([P, CW], bf16, tag="w1c")
                nc.sync.dma_start(out=wch[:], in_=w1_d[:, ct * CW:(ct + 1) * CW])
                pb = lpsum.tile([P, RS * B], f32, tag="lin1")
                nc.tensor.matmul(pb[:], lhsT=b1[:, ct * P:(ct + 1) * P],
                                 rhs=delta[:], start=True, stop=False,
                                 skip_group_check=True)
                for r in range(RS):
                    for kh in range(2):
                        nc.tensor.matmul(
                            pb[:, r * B:(r + 1) * B],
                            lhsT=wch[:, (r * 2 + kh) * P:(r * 2 + kh + 1) * P],
                            rhs=instrT[:, kh * B:(kh + 1) * B],
                            start=False, stop=(r == RS - 1 and kh == 1),
                            skip_group_check=True,
                        )
                out_ap = (f1_sb[:, ct * B * RS:(ct + 1) * B * RS]
                          .rearrange("p (b r) -> p r b", b=B))
                nc.scalar.activation(out=out_ap, in_=pb[:], func=AFT.Lrelu,
                                     alpha=0.01)
            lin1_psum.__exit__(None, None, None)

            # ---- conv + fused BN-shift + channel max ----------------------
            # Orientation: out[r, hw] with the per-sample filter f1T as the
            # stationary (LDW amortized over N=512 moving cols).  4 samples
            # run concurrently in the four 32-col PE groups (tile_position is
            # inferred from the PSUM slice base partition).  The cross-
            # partition max over each sample's 32 r-rows is a DVE
            # transpose-reduce (32x32 reshape front-end).
            vout = vpool.tile([P, P], f32)  # [j*32+q, g*32 + n*16 + blk]
            conv_psum = tc.tile_pool(name="cpsum", bufs=4, space="PSUM")
            cpsum = conv_psum.__enter__()
            fbs = {}
            for b in range(B):
                fbs[b] = fpool.tile([P, 8 * HW2], bf16, tag="feat",
                                    name=f"fb{b}")
                nc.sync.dma_start(
                    out=fbs[b][:].rearrange("p (kc hw) -> p kc hw", kc=8),
                    in_=feat_d[b])
            NH = HW2 // 2  # 512
            for g in range(4):
                for n in range(2):
                    pc = cpsum.tile([P, NH], f32, tag="conv")
                    nc.tensor.matmul(pc[:], lhsT=tpat[:], rhs=ones_row[:],
                                     start=True, stop=False,
                                     skip_group_check=True)
                    for kc in range(8):
                        for j in range(4):
                            b = 4 * g + j
                            nc.tensor.matmul(
                                pc[j * RS:(j + 1) * RS, :],
                                lhsT=f1_sb[:, kc * B * RS + b * RS:
                                           kc * B * RS + (b + 1) * RS],
                                rhs=fbs[b][:, kc * HW2 + n * NH:
                                           kc * HW2 + (n + 1) * NH],
                                start=False, stop=(kc == 7),
                                skip_group_check=True,
                                tile_position=(0, j * RS),
                            )
                    # DVE transpose-reduce can't read PSUM; stage via ACT copy
                    cp = tmppool.tile([P, NH], f32, tag="convcp")
                    nc.scalar.activation(out=cp[:], in_=pc[:], func=AFT.Copy)
                    nc.vector.tensor_reduce(
                        out=vout[:, g * 32 + n * 16: g * 32 + (n + 1) * 16],
                        in_=cp[:].rearrange("p (blk q) -> p blk q", q=32),
                        axis=AX.X, op=AluOpType.max, apply_transpose=True)
            conv_psum.__exit__(None, None, None)

            # contiguous store; host decodes the (j,q),(g,n,blk) layout
            nc.sync.dma_start(out=out_d, in_=vout[:])

    _split_excess_waits(nc)
    return nc


# ---------------------------------------------------------------------------
def _prep_inputs(feature, instruction_idx, instruction_length, emb_table,
                 w_ih, w_hh, b_ih, b_hh, e2d_w, e2d_b,
                 lin1_w, lin1_b, bn_gamma, bn_beta, bn_mean, bn_var):
    """Host-side layout/dtype prep. Returns (in_maps, slots, T0)."""
    f32 = np.float32

    def to_bf(x):
        return np.ascontiguousarray(x.astype(BF16))

    feature = np.asarray(feature, f32)
    emb_table = np.asarray(emb_table, f32)
    idx = np.asarray(instruction_idx)
    lengths = np.asarray(instruction_length).astype(np.int64)
    slots = [int(max(l, 1) - 1) for l in lengths]

    # feature (b, c_in, kc, hw): per-partition data contiguous (16KB) so the
    # DMA uses 16KB descriptors instead of 2KB
    feat = to_bf(feature.reshape(B, 8, P, HW2).transpose(0, 2, 1, 3))

    # embeds transposed: [p, (ke, t*b)]
    emb = emb_table[idx]                       # [B, S, E]
    embT = emb.transpose(2, 1, 0).reshape(2, P, S * B)
    embT = to_bf(embT.transpose(1, 0, 2).reshape(P, 2 * S * B))

    def wtiles(w, kt, mt):
        # w: [out, in] -> lhsT tiles arr[p, (k, m, col)] with lhsT=w.T tile
        wt = np.asarray(w, f32).T  # [in, out]
        a = wt.reshape(kt, P, mt, P).transpose(1, 0, 2, 3)
        return to_bf(a.reshape(P, kt * mt * P))

    # tanh(g) computed as 2*sigmoid(2g)-1: scale the g-gate rows (512:768)
    # by 2 so one big sigmoid covers all four gates.
    gsc = np.ones((4 * HID, 1), f32)
    gsc[2 * HID:3 * HID] = 2.0
    wihT = wtiles(np.asarray(w_ih, f32) * gsc, 2, 8)
    whhT = wtiles(np.asarray(w_hh, f32) * gsc, 2, 8)
    e2dT = wtiles(e2d_w, 2, 2)

    bg = ((np.asarray(b_ih, f32) + np.asarray(b_hh, f32)) * gsc[:, 0]) \
        .reshape(8, P).T.copy()
    e2db = np.asarray(e2d_b, f32).reshape(2, P).T.copy()

    s = np.asarray(bn_gamma, f32) / np.sqrt(np.asarray(bn_var, f32) + BN_EPS)
    tsh = np.asarray(bn_beta, f32) - np.asarray(bn_mean, f32) * s
    T0 = float(tsh.max())

    w1s = np.asarray(lin1_w, f32).reshape(R, C, HID) * s[:, None, None]
    b1s = np.asarray(lin1_b, f32).reshape(R, C) * s[:, None]

    delta = np.repeat(np.eye(RS, dtype=f32), B, axis=1)  # [32, 512]
    eye = np.eye(P, dtype=f32)
    ones32 = np.ones((RS, P), f32)

    def pad128(a):
        out = np.zeros((P, a.shape[1]), f32)
        out[:a.shape[0]] = a
        return out

    biasf = np.concatenate([bg, e2db], axis=1).astype(f32)  # [128, 10]
    biasf = np.ascontiguousarray(biasf)

    in_maps = []
    for k in range(N_CORES):
        rsl = slice(k * RS, (k + 1) * RS)
        wsl = w1s[rsl]                          # [32, 1024, 256] (r, c, h)
        # tiles (ct, r, kh): arr[p, ...] = w.T[kh*128+p, r, ct*128+col]
        ws = wsl.transpose(2, 1, 0)             # [h, c, r]
        a = (ws.reshape(2, P, 8, P, RS)         # [kh, p, ct, col, r]
             .transpose(1, 2, 4, 0, 3)          # [p, ct, r, kh, col]
             .reshape(P, 8 * RS * 2 * P))
        b1c = b1s[rsl].reshape(RS, 8, P).reshape(RS, 8 * P)  # (r, (ct, c))
        # conv BN-shift injection: out[p,:] += t[p%32] via rank-1 matmul
        tpat = np.zeros((RS, P), f32)
        tpat[0] = np.tile(tsh[rsl], 4)
        ones_row = np.zeros((RS, HW2 // 2), f32)
        ones_row[0] = 1.0
        cpack = np.concatenate(
            [embT.astype(f32), wihT.astype(f32), whhT.astype(f32),
             e2dT.astype(f32), eye, pad128(b1c), pad128(delta),
             pad128(tpat), pad128(ones_row)], axis=1)
        in_maps.append(dict(feat=feat, cpack=to_bf(cpack), biasf=biasf,
                            w1T=to_bf(a)))
    return in_maps, slots, T0


_cache = {}


def _run(inputs, trace=False):
    (in_maps, slots, T0) = _prep_inputs(
        inputs["feature"], inputs["instruction_idx"],
        inputs["instruction_length"], inputs["emb_table"],
        inputs["w_ih"], inputs["w_hh"], inputs["b_ih"], inputs["b_hh"],
        inputs["e2d_w"], inputs["e2d_b"], inputs["lin1_w"], inputs["lin1_b"],
        inputs["bn_gamma"], inputs["bn_beta"], inputs["bn_mean"],
        inputs["bn_var"])

    key = tuple(slots)
    if key not in _cache:
        _cache[key] = _build_program(slots)
    nc = _cache[key]

    kw = {}
    if trace:
        kw = dict(trace=True, trace_cores=list(range(N_CORES)))
    res = run_bass_kernel_spmd(nc, in_maps, list(range(N_CORES)), **kw)
    parts = np.stack([np.asarray(res.results[i]["part_out"], np.float32)
                      for i in range(N_CORES)])   # [8, p=(j,q), col=(g,n,blk)]
    v = parts.reshape(N_CORES, 4, 32, 4, 2, 16)   # [core, j, q, g, n, blk]
    v = v.transpose(0, 3, 1, 4, 5, 2)             # [core, g, j, n, blk, q]
    single = v.reshape(N_CORES, B, HW2).max(axis=0)
    single = np.maximum(single, T0)
    out = np.clip(single, 0.0, 1.0).reshape(B, 32, 32).astype(np.float32)
    return out, res


def kernel(**inputs) -> np.ndarray:
    out, _ = _run(inputs, trace=False)
    return out


def kernel_traced(**inputs):
    out, res = _run(inputs, trace=True)
    return out, res



# revision 17
# speedup vs baseline: 1.0663x; 1.0663x over previous
"""Trainium2 Bass kernel for nn_Map_79748952752358 (dense_cnn).

Pipeline: LSTM encoder (40 steps) -> e2d projection -> big linear
(lin1: 256 -> 262144) -> per-sample dynamic 1x1 conv over feature
[1024, 32x32] -> BN(eval) -> channel-max -> clip.

Sharding (v2):
  - LSTM + e2d replicated on all 8 cores (serial recurrence, tiny state).
  - lin1 tensor-sharded over R: core k computes filter rows r in
    [32k, 32k+32) for ALL 16 samples (1/8 of the 134MB W1 streams per
    core; prefetched into SBUF during the LSTM so lin1 is PE-bound).
  - One AllToAll redistributes filters so core k holds the FULL
    [256, 1024] filter block for ITS two samples (2k, 2k+1).
  - conv batch-sharded: each core convolves its 2 samples with full
    R=256 (M=128 matmuls, feature slice only 4.2MB/core).
  - per-core output: channel-max partials [128, 64]; host combines the
    four 32-row groups, applies the BN-shift floor T0 and the clip.

Key math folds (exact, host side):
  - BN scale s=gamma/sqrt(var+eps) > 0 folded into lin1 weights/bias.
  - relu(x)+t maxed over r == max(max_r(x+t), max_r(t)); +t injected
    into conv PSUM via a rank-1 matmul, max_r(t)=T0 applied on host.
"""

import numpy as np
import ml_dtypes

import concourse.bass as bass
import concourse.mybir as mybir
from concourse import tile
from concourse.tile import ScopedClock
from concourse.alu_op_type import AluOpType
from concourse.bass_utils import run_bass_kernel_spmd

BF16 = ml_dtypes.bfloat16

B, S, V, E, HID = 16, 40, 1004, 256, 256
C, R, HW2 = 1024, 256, 1024
BN_EPS = 1e-5
N_CORES = 8
RS = R // N_CORES   # 32 r-rows per core (lin1 shard)
BS = B // N_CORES   # 2 samples per core (conv shard)
P = 128

AFT = mybir.ActivationFunctionType
AX = mybir.AxisListType


# ---------------------------------------------------------------------------
# Tile tail-drain patch: this walrus build accepts fewer sem waits per
# TPB_CTRL instruction than Tile's exit drain accumulates; split them into
# single-wait SP nops.
_drain_patched = False


def _patch_tile_drain():
    global _drain_patched
    if _drain_patched:
        return
    _drain_patched = True

    def _patched(self, tick_clock, wait_clock):
        nc = self.nc
        probe = nc.sync.nop(nofuse=True, hint="drain_wait_split")
        wait_clock.add_sem_waits(
            probe.ins, ScopedClock({None: tick_clock.global_clock})
        )
        si = probe.ins.sync_info
        waits = list(si.on_wait or []) if si is not None else []
        if len(waits) > 1:
            si.on_wait = waits[:1]
            for w in waits[1:]:
                n = nc.sync.nop(nofuse=True, hint="drain_wait_split")
                nsi = n.ins.sync_info
                if nsi is None:
                    import bass_rust

                    n.ins.sync_info = bass_rust.SyncInfo(on_wait=[w], on_update=[])
                else:
                    nsi.on_wait = [w]
        nc.sync.drain()
        nc.all_engine_barrier()
        assert self.sems is not None
        popped = nc._tile_sem_poison_stack.pop()
        assert popped is self._sem_poison
        nc.clear_and_free_semaphores(list(self.sems.allocated().values()))
        nc.all_engine_barrier()

    tile.TileContext._drain_and_barrier = _patched


_ws_counter = [0]


def _split_excess_waits(nc, limit=1):
    """Walrus on this image rejects instructions with more than ~2 sem waits.
    Move excess waits onto same-engine EventSemaphore carriers inserted just
    before the offending instruction (same per-engine stream order, identical
    blocking semantics)."""
    import bass_rust

    for fn in nc.m.functions:
        for bb in fn.blocks:
            out = []
            for inst in bb.instructions:
                si = inst.sync_info
                waits = list(si.on_wait or []) if si is not None else []
                if len(waits) > limit:
                    for w in waits[:-limit]:
                        _ws_counter[0] += 1
                        carrier = mybir.InstEventSemaphore(
                            name=f"I-waitsplit-{_ws_counter[0]}",
                            opcode="EventSemaphore",
                            engine=inst.engine,
                            sync_info=bass_rust.SyncInfo(
                                on_wait=[w], on_update=[]),
                        )
                        out.append(carrier)
                    si.on_wait = waits[-limit:]
                out.append(inst)
            bb.instructions = out


# ---------------------------------------------------------------------------
def _build_program(slots):
    """Build the SPMD Bass program. `slots[b]` = length[b]-1, the LSTM step
    whose hidden state is each sample's final state (compile-time constants).
    """
    _patch_tile_drain()
    nc = bass.Bass("TRN2", target_bir_lowering=False, debug=False,
                   num_devices=N_CORES)
    dt = mybir.dt
    f32, bf16 = dt.float32, dt.bfloat16

    def din(name, shape, d=bf16):
        return nc.dram_tensor(name, shape, d, kind="ExternalInput").ap()

    # feature slice for this core's 2 samples: (b, c_in_chunk, kc, hw)
    feat_d = din("feat", [BS, P, 8, HW2])
    # one packed bf16 constant block: embT | wihT | whhT | e2dT | eye |
    # b1 | delta | tpat2 | ones_row  (partition-padded blocks)
    PK = [2 * S * B, 16 * P, 16 * P, 4 * P, P, 8 * P, RS * B, 2 * P, HW2 // 2]
    pack_d = din("cpack", [P, sum(PK)])
    biasf_d = din("biasf", [P, 10], f32)          # biasg(8) | e2db(2)
    w1_d = din("w1T", [P, 8 * RS * 2 * P])        # tiles (ct, r, kh)

    out_d = nc.dram_tensor("part_out", [P, BS * 2 * 2 * 16], f32,
                           kind="ExternalOutput").ap()

    with tile.TileContext(nc) as tc:
        with (
            tc.tile_pool(name="const", bufs=1) as cpool,
            tc.tile_pool(name="xg", bufs=1) as xgpool,
            tc.tile_pool(name="hist", bufs=1) as hpool,
            tc.tile_pool(name="gs", bufs=2) as gspool,
            tc.tile_pool(name="cell", bufs=1) as cellpool,
            tc.tile_pool(name="tmp", bufs=4) as tmppool,
            tc.tile_pool(name="w1c", bufs=6) as w1pool,
            tc.tile_pool(name="f1", bufs=1) as f1pool,
            tc.tile_pool(name="feat", bufs=2) as fpool,
            tc.tile_pool(name="f1c", bufs=1) as f1cpool,
            tc.tile_pool(name="rmax", bufs=4) as rmpool,
            tc.tile_pool(name="vout", bufs=1) as vpool,
            tc.tile_pool(name="a2a", bufs=1, space="DRAM") as dpool,
        ):
            pack = cpool.tile([P, sum(PK)], bf16, tag="cpack")
            nc.sync.dma_start(out=pack[:], in_=pack_d)
            biasf = cpool.tile([P, 10], f32, tag="biasf")
            nc.sync.dma_start(out=biasf[:], in_=biasf_d)

            off = np.cumsum([0] + PK)
            embT = pack[:, off[0]:off[1]]
            wih = pack[:, off[1]:off[2]]
            whh = pack[:, off[2]:off[3]]
            e2dT = pack[:, off[3]:off[4]]
            eye = pack[:, off[4]:off[5]]
            b1 = pack[0:RS, off[5]:off[6]]
            delta = pack[0:RS, off[6]:off[7]]
            tpat2 = pack[0:RS, off[7]:off[8]]     # row0 = tsh (natural order)
            ones_row = pack[0:RS, off[8]:off[9]]  # row0 = 1
            biasg = biasf[:, 0:8]
            e2db = biasf[:, 8:10]

            # ---- prefetched loads: w1 chunks (6 of 8 resident), feature ----
            CW = RS * 2 * P  # w1 chunk cols per ct

            def w1_fetch(ct):
                wch = w1pool.tile([P, CW], bf16, tag="w1c", name=f"wch{ct}")
                nc.sync.dma_start(out=wch[:], in_=w1_d[:, ct * CW:(ct + 1) * CW])
                return wch

            wchs = []
            for ct in range(6):
                wchs.append(w1_fetch(ct))
                if ct == 3:
                    # feature slice DMA sits between w1 chunk 3 and 4 so it
                    # is in flight well before the conv needs it.
                    fbs = []
                    for b in range(BS):
                        fb = fpool.tile([P, 8 * HW2], bf16, tag="feat",
                                        name=f"fb{b}")
                        nc.sync.dma_start(
                            out=fb[:].rearrange("p (kc hw) -> p kc hw", kc=8),
                            in_=feat_d[b])
                        fbs.append(fb)

            # ---- Stage A: xg = w_ih @ x_t for all steps (+ gate bias) ----
            xg_s = xgpool.tile([P, 8 * S * B], bf16)
            NCH = 320  # psum N-chunk: 20 steps x 16
            with tc.tile_pool(name="xpsum", bufs=2, space="PSUM") as xpsum:
                for m in range(8):
                    for n in range(2):
                        ps = xpsum.tile([P, NCH], f32, tag="xg")
                        for ke in range(2):
                            nc.tensor.matmul(
                                ps[:],
                                lhsT=wih[:, (ke * 8 + m) * P:(ke * 8 + m + 1) * P],
                                rhs=embT[:, ke * S * B + n * NCH: ke * S * B + (n + 1) * NCH],
                                start=(ke == 0), stop=(ke == 1),
                            )
                        nc.scalar.activation(
                            out=xg_s[:, m * S * B + n * NCH: m * S * B + (n + 1) * NCH],
                            in_=ps[:], func=AFT.Identity, bias=biasg[:, m:m + 1],
                        )

            # ---- Stage B: LSTM recurrence (layout: gate-dim on partitions) --
            S_eff = max(slots) + 1
            hist = hpool.tile([P, S * 2 * B], bf16)   # (t, kh, b)
            c_s = cellpool.tile([P, 2 * B], f32)      # (kh, b)
            xg_r = xg_s[:].rearrange("p (m t b) -> p m t b", m=8, t=S)
            lstm_psum = tc.tile_pool(name="gpsum", bufs=2, space="PSUM")
            gpsum = lstm_psum.__enter__()
            for t in range(S_eff):
                gp = gpsum.tile([P, P], f32, tag="gates")
                nc.tensor.matmul(gp[:], lhsT=eye[:], rhs=xg_r[:, :, t, :],
                                 start=True, stop=(t == 0))
                if t > 0:
                    for m in range(8):
                        for kh in range(2):
                            nc.tensor.matmul(
                                gp[:, m * B:(m + 1) * B],
                                lhsT=whh[:, (kh * 8 + m) * P:(kh * 8 + m + 1) * P],
                                rhs=hist[:, (t - 1) * 2 * B + kh * B:
                                         (t - 1) * 2 * B + (kh + 1) * B],
                                start=False, stop=(m == 7 and kh == 1),
                                skip_group_check=True,
                            )
                gs = gspool.tile([P, P], f32, tag="gs")
                # cols (m,b): i=0:32, f=32:64, g=64:96, o=96:128
                # one sigmoid for all gates; tanh(g)=2*sig(2g)-1 (g-rows
                # pre-scaled by 2 on host; fp32 to avoid rounding blowup)
                nc.scalar.activation(out=gs[:], in_=gp[:], func=AFT.Sigmoid)
                tg = tmppool.tile([P, 2 * B], f32, tag="tg")
                nc.vector.tensor_scalar(tg[:], gs[:, 64:96], 2.0, -1.0,
                                        AluOpType.mult, AluOpType.add)
                t1 = tmppool.tile([P, 2 * B], f32, tag="t1")
                nc.vector.tensor_tensor(t1[:], gs[:, 0:32], tg[:],
                                        AluOpType.mult)
                if t == 0:
                    nc.vector.tensor_copy(c_s[:], t1[:])
                else:
                    t2 = tmppool.tile([P, 2 * B], f32, tag="t2")
                    nc.vector.tensor_tensor(t2[:], gs[:, 32:64], c_s[:],
                                            AluOpType.mult)
                    nc.vector.tensor_tensor(c_s[:], t1[:], t2[:], AluOpType.add)
                th = tmppool.tile([P, 2 * B], bf16, tag="th")
                nc.scalar.activation(out=th[:], in_=c_s[:], func=AFT.Tanh)
                nc.vector.tensor_tensor(
                    hist[:, t * 2 * B:(t + 1) * 2 * B],
                    gs[:, 96:128], th[:], AluOpType.mult)

            # ---- capture final h per sample (slots known at build time) ----
            h_fin = cellpool.tile([P, 2 * B], bf16, tag="hfin")  # (kh, b)
            hf_r = h_fin[:].rearrange("p (k b) -> p b k", k=2)
            for b in range(B):
                src = hist[:, slots[b] * 2 * B:(slots[b] + 1) * 2 * B]
                nc.vector.tensor_copy(
                    hf_r[:, b], src.rearrange("p (k b) -> p b k", k=2)[:, b])

            # ---- e2d projection: instrT = tanh(e2d_w @ h + b) -------------
            instrT = cellpool.tile([P, 2 * B], bf16, tag="instrT")  # (kh, b)
            for m in range(2):
                pe2 = gpsum.tile([P, B], f32, tag="e2d")
                for kh in range(2):
                    nc.tensor.matmul(
                        pe2[:],
                        lhsT=e2dT[:, (kh * 2 + m) * P:(kh * 2 + m + 1) * P],
                        rhs=h_fin[:, kh * B:(kh + 1) * B],
                        start=(kh == 0), stop=(kh == 1),
                    )
                nc.scalar.activation(out=instrT[:, m * B:(m + 1) * B],
                                     in_=pe2[:], func=AFT.Tanh,
                                     bias=e2db[:, m:m + 1])
            lstm_psum.__exit__(None, None, None)

            # ---- lin1 (c-chunk slice): core k computes f1[all r, c-chunk k]
            # for all 16 samples.  PSUM partitions = c_local; loop over 8
            # r-tiles of 32.  f1_sb cols = b*256 + r  (b-major so the
            # per-dest a2a slices are contiguous).
            f1_sb = f1pool.tile([P, B * R], bf16)
            a2a_in = dpool.tile([N_CORES * P, BS * R], bf16)
            a2a_out = dpool.tile([N_CORES * P, BS * R], bf16)
            f1v = f1_sb[:].rearrange("p (b r) -> p r b", b=B)
            lin1_psum = tc.tile_pool(name="lpsum", bufs=4, space="PSUM")
            lpsum = lin1_psum.__enter__()
            for rt in range(8):
                wch = wchs[rt]
                if rt + 6 < 8:
                    wchs.append(w1_fetch(rt + 6))
                pb = lpsum.tile([P, RS * B], f32, tag="lin1")
                nc.tensor.matmul(pb[:], lhsT=b1[:, rt * P:(rt + 1) * P],
                                 rhs=delta[:], start=True, stop=False,
                                 skip_group_check=True)
                for r in range(RS):
                    for kh in range(2):
                        nc.tensor.matmul(
                            pb[:, r * B:(r + 1) * B],
                            lhsT=wch[:, (r * 2 + kh) * P:(r * 2 + kh + 1) * P],
                            rhs=instrT[:, kh * B:(kh + 1) * B],
                            start=False, stop=(r == RS - 1 and kh == 1),
                            skip_group_check=True,
                        )
                # pb cols (r, b) -> f1_sb strided (b, r-slice rt)
                nc.scalar.activation(out=f1v[:, rt * RS:(rt + 1) * RS],
                                     in_=pb[:], func=AFT.Lrelu, alpha=0.01)
            lin1_psum.__exit__(None, None, None)

            # ---- AllToAll: ship my c-chunk of dest-core samples -----------
            # in shard j (rows 128j..): [c_local, (b2)(r256)] = f1_sb cols
            # [512j : 512j+512] (contiguous per partition)
            a2a_iv = a2a_in[:].rearrange("(j c) x -> c j x", j=N_CORES)
            nc.gpsimd.dma_start(
                out=a2a_iv,
                in_=f1_sb[:].rearrange("p (j x) -> p j x", j=N_CORES))
            nc.gpsimd.collective_compute(
                "AllToAll", AluOpType.bypass,
                replica_groups=[list(range(N_CORES))],
                ins=[a2a_in[:].opt()], outs=[a2a_out[:].opt()],
            )
            # gather: shard i = c-chunk i of my 2 samples, all 256 r.
            # f1c cols = b*2048 + m*1024 + kc*128 + r_sub: conv lhsT tiles
            # [c, 128 r] are contiguous slices.
            f1c = f1cpool.tile([P, BS * 2 * 8 * P], bf16)
            f1cv = f1c[:].rearrange("p (b m kc r) -> p kc b m r",
                                    b=BS, m=2, kc=8)
            a2a_ov = a2a_out[:].rearrange("(kc c) (b m r) -> kc c b m r",
                                          kc=N_CORES, b=BS, m=2)
            for kc in range(8):
                nc.gpsimd.dma_start(out=f1cv[:, kc], in_=a2a_ov[kc])

            # ---- conv + fused BN-shift + channel max ----------------------
            # out[r-tile, hw]: lhsT = f1c tile [c, (i4, r)] (strided cols),
            # rhs = feature [c, hw-chunk].  PSUM tile per (b, n, m); each is
            # ACT-copied to SBUF (m0's copy hides under m1's matmuls) and
            # transpose-reduced to per-32-row-group maxes; host folds m/j.
            vout = vpool.tile([P, BS * 2 * 2 * 16], f32)  # [(j,q),(b,n,m,blk)]
            f1t = f1c[:].rearrange("p (b m kc r) -> p b m kc r",
                                   b=BS, m=2, kc=8)
            NH = HW2 // 2  # 512
            conv_psum = tc.tile_pool(name="cpsum", bufs=6, space="PSUM")
            cpsum = conv_psum.__enter__()
            for b in range(BS):
                for n in range(2):
                    for m in range(2):
                        pc = cpsum.tile([P, NH], f32, tag="conv")
                        nc.tensor.matmul(
                            pc[:], lhsT=tpat2[:, m * P:(m + 1) * P],
                            rhs=ones_row[:, 0:NH],
                            start=True, stop=False, skip_group_check=True)
                        for kc in range(8):
                            nc.tensor.matmul(
                                pc[:],
                                lhsT=f1t[:, b, m, kc],
                                rhs=fbs[b][:, kc * HW2 + n * NH:
                                           kc * HW2 + (n + 1) * NH],
                                start=False, stop=(kc == 7),
                                skip_group_check=True,
                            )
                        cp = rmpool.tile([P, NH], f32, tag="convcp")
                        nc.scalar.activation(out=cp[:], in_=pc[:],
                                             func=AFT.Copy)
                        col = ((b * 2 + n) * 2 + m) * 16
                        nc.vector.tensor_reduce(
                            out=vout[:, col:col + 16],
                            in_=cp[:].rearrange("p (blk q) -> p blk q", q=32),
                            axis=AX.X, op=AluOpType.max, apply_transpose=True)
            conv_psum.__exit__(None, None, None)

            # contiguous store; host decodes the (j,q),(b,n,blk) layout
            nc.sync.dma_start(out=out_d, in_=vout[:])

    _split_excess_waits(nc)
    return nc


# ---------------------------------------------------------------------------
def _prep_inputs(feature, instruction_idx, instruction_length, emb_table,
                 w_ih, w_hh, b_ih, b_hh, e2d_w, e2d_b,
                 lin1_w, lin1_b, bn_gamma, bn_beta, bn_mean, bn_var):
    """Host-side layout/dtype prep. Returns (in_maps, slots, T0)."""
    f32 = np.float32

    def to_bf(x):
        return np.ascontiguousarray(x.astype(BF16))

    feature = np.asarray(feature, f32)
    emb_table = np.asarray(emb_table, f32)
    idx = np.asarray(instruction_idx)
    lengths = np.asarray(instruction_length).astype(np.int64)
    slots = [int(max(l, 1) - 1) for l in lengths]

    # feature (b, c_in, kc, hw): per-partition data contiguous (16KB)
    feat = to_bf(feature.reshape(B, 8, P, HW2).transpose(0, 2, 1, 3))

    # embeds transposed: [p, (ke, t*b)]
    emb = emb_table[idx]                       # [B, S, E]
    embT = emb.transpose(2, 1, 0).reshape(2, P, S * B)
    embT = to_bf(embT.transpose(1, 0, 2).reshape(P, 2 * S * B))

    def wtiles(w, kt, mt):
        # w: [out, in] -> lhsT tiles arr[p, (k, m, col)] with lhsT=w.T tile
        wt = np.asarray(w, f32).T  # [in, out]
        a = wt.reshape(kt, P, mt, P).transpose(1, 0, 2, 3)
        return to_bf(a.reshape(P, kt * mt * P))

    # tanh(g) computed as 2*sigmoid(2g)-1: scale the g-gate rows (512:768)
    # by 2 so one big sigmoid covers all four gates.
    gsc = np.ones((4 * HID, 1), f32)
    gsc[2 * HID:3 * HID] = 2.0
    wihT = wtiles(np.asarray(w_ih, f32) * gsc, 2, 8)
    whhT = wtiles(np.asarray(w_hh, f32) * gsc, 2, 8)
    e2dT = wtiles(e2d_w, 2, 2)

    bg = ((np.asarray(b_ih, f32) + np.asarray(b_hh, f32)) * gsc[:, 0]) \
        .reshape(8, P).T.copy()
    e2db = np.asarray(e2d_b, f32).reshape(2, P).T.copy()

    s = np.asarray(bn_gamma, f32) / np.sqrt(np.asarray(bn_var, f32) + BN_EPS)
    tsh = np.asarray(bn_beta, f32) - np.asarray(bn_mean, f32) * s
    T0 = float(tsh.max())

    w1s = np.asarray(lin1_w, f32).reshape(R, C, HID) * s[:, None, None]
    b1s = np.asarray(lin1_b, f32).reshape(R, C) * s[:, None]

    delta = np.repeat(np.eye(RS, dtype=f32), B, axis=1)  # [32, 512]
    eye = np.eye(P, dtype=f32)

    def pad128(a):
        out = np.zeros((P, a.shape[1]), f32)
        out[:a.shape[0]] = a
        return out

    biasf = np.concatenate([bg, e2db], axis=1).astype(f32)  # [128, 10]
    biasf = np.ascontiguousarray(biasf)

    # conv BN-shift injection: rank-1 matmul row (natural r order: the conv
    # r-tile m holds global r = m*128 + i4*32 + r_sub)
    tpat2 = np.zeros((RS, 2 * P), f32)
    tpat2[0] = tsh
    ones_row = np.zeros((RS, HW2 // 2), f32)
    ones_row[0] = 1.0

    in_maps = []
    for k in range(N_CORES):
        csl = slice(k * P, (k + 1) * P)
        wsl = w1s[:, csl]                       # [256, 128, 256] (r, c, h)
        # tiles (rt, r_local, kh): arr[p, ...] = W'T[kh*128+p, rt*32+rl, c]
        ws = wsl.transpose(2, 0, 1)             # [h, r, c]
        a = (ws.reshape(2, P, 8, RS, P)         # [kh, p, rt, rl, c]
             .transpose(1, 2, 3, 0, 4)          # [p, rt, rl, kh, c]
             .reshape(P, 8 * RS * 2 * P))
        # b1 inject tile per rt: [rl, (rt, c)]
        b1c = (b1s[:, csl].reshape(8, RS, P)    # [rt, rl, c]
               .transpose(1, 0, 2).reshape(RS, 8 * P))
        cpack = np.concatenate(
            [embT.astype(f32), wihT.astype(f32), whhT.astype(f32),
             e2dT.astype(f32), eye, pad128(b1c), pad128(delta),
             pad128(tpat2), pad128(ones_row)], axis=1)
        in_maps.append(dict(feat=feat[2 * k:2 * k + 2], cpack=to_bf(cpack),
                            biasf=biasf, w1T=to_bf(a)))
    return in_maps, slots, T0


_cache = {}


def _run(inputs, trace=False):
    (in_maps, slots, T0) = _prep_inputs(
        inputs["feature"], inputs["instruction_idx"],
        inputs["instruction_length"], inputs["emb_table"],
        inputs["w_ih"], inputs["w_hh"], inputs["b_ih"], inputs["b_hh"],
        inputs["e2d_w"], inputs["e2d_b"], inputs["lin1_w"], inputs["lin1_b"],
        inputs["bn_gamma"], inputs["bn_beta"], inputs["bn_mean"],
        inputs["bn_var"])

    key = tuple(slots)
    if key not in _cache:
        _cache[key] = _build_program(slots)
    nc = _cache[key]

    kw = {}
    if trace:
        kw = dict(trace=True, trace_cores=list(range(N_CORES)))
    res = run_bass_kernel_spmd(nc, in_maps, list(range(N_CORES)), **kw)
    # per-core out: [128=(j,q), 128=(b,n,m,blk)]; sample = 2*core + b,
    # hw = n*512 + blk*32 + q, value = max over 32-row group j of r-tile m.
    parts = np.stack([np.asarray(res.results[i]["part_out"], np.float32)
                      for i in range(N_CORES)])      # [8, 128, 128]
    v = parts.reshape(N_CORES, 4, 32, BS, 2, 2, 16)  # [c, j, q, b, n, m, blk]
    v = v.max(axis=(1, 5))                           # [core, q, b, n, blk]
    v = v.transpose(0, 2, 3, 4, 1)                   # [core, b, n, blk, q]
    single = v.reshape(B, HW2)
    single = np.maximum(single, T0)
    out = np.clip(single, 0.0, 1.0).reshape(B, 32, 32).astype(np.float32)
    return out, res


def kernel(**inputs) -> np.ndarray:
    out, _ = _run(inputs, trace=False)
    return out


def kernel_traced(**inputs):
    out, res = _run(inputs, trace=True)
    return out, res
